# revision 1
# baseline (speedup 1.0000x reference)
"""CRF NLL kernel for Trainium2 (8 NeuronCores, SPMD-replicated).

Math: the reference forward algorithm
    alpha_t[j] = logsumexp_i(alpha_{t-1}[i] + T[i,j]) + em_t[j]
runs in LINEAR space with a host-estimated per-timestep rescale c_t:
    v_t = (v_{t-1} @ expT) * exp(em_t - c_t)
so  log_den = log(sum(v_4095)) - log(1024) + sum_t c_t.  The c_t table
(log of the column-mean-weighted emission partition) tracks the true
per-step growth so well that v stays within ~2x of 1.0 for the whole
4095-step scan -- no logsumexp, max, renormalization or overflow
handling is needed, and v can be held in fp8.

Per scan step on the PE: expT lives in SBUF as fp8e4 [128, 8, 1024]
and v as fp8e4 [128, 8(pairs), 16]; 8 DoubleRow matmuls (2 fp8
contraction rows per cell, 0.5 cycles/output element) compute
v @ expT into PSUM in ~850ns.  The row vector returns to partition
layout via 8 partition-aligned single-row copies (DVE/ACT split) into
two bf16 staging tiles and 2 PE transposes; a DVE multiply applies the
prefetched exp(em_t - c_t) tile and re-quantizes v to fp8.

The emission table is transposed host-side; per-timestep rows are
gathered on-device with indirect DMA.  The log numerator is computed
on-device with the same gathers plus iota/compare/mask/reduce.  The
scan is inherently sequential and cross-core collectives have a ~60us
floor, so the kernel is replicated on all 8 cores; core 0's output is
returned.  Validated end-to-end error of this scheme vs the fp32
reference: ~1e-5 relative.
"""
import sys

sys.path.insert(0, '/opt/trn_rl_repo')

from contextlib import ExitStack

import numpy as np

import concourse.bass as bass
import concourse.mybir as mybir
import concourse.tile as tile
from concourse.bass import Bass
from concourse.bass_utils import run_bass_kernel_spmd
from concourse.masks import make_identity

N_STATES = 1024
N_OBS = 32000
SB = 8            # state blocks of 128
P = 128
UH = 15           # scan steps per half-body

_F32 = mybir.dt.float32
_F32R = mybir.dt.float32r
_BF16 = mybir.dt.bfloat16
_FP8 = mybir.dt.float8e4
_I32 = mybir.dt.int32
LOG1024 = float(np.log(1024.0))


def _split_multi_sync(nc):
    """This walrus build rejects >1 sync wait / update per instruction.
    Move extras onto same-engine NoOps (engine queues are in-order)."""
    n = 0
    for f in nc.m.functions:
        for bb in f.blocks:
            newl = []
            changed = False
            for inst in bb.instructions:
                si = inst.sync_info
                waits = list(si.on_wait or []) if si is not None else []
                updates = list(si.on_update or []) if si is not None else []
                pre = []
                post = []
                if len(waits) > 1:
                    for k, w in enumerate(waits[:-1]):
                        nop = mybir.InstNoOp(name=f"{inst.name}-wsp{k}",
                                             engine=inst.engine)
                        nop.sync_info = mybir.SyncInfo(on_wait=[w], on_update=[])
                        pre.append(nop)
                    waits = waits[-1:]
                if len(updates) > 1:
                    for k, u in enumerate(updates[1:]):
                        nop = mybir.InstNoOp(name=f"{inst.name}-usp{k}",
                                             engine=inst.engine)
                        nop.sync_info = mybir.SyncInfo(on_wait=[], on_update=[u])
                        post.append(nop)
                    updates = updates[:1]
                if pre or post:
                    changed = True
                    inst.sync_info = mybir.SyncInfo(on_wait=waits, on_update=updates)
                    n += len(pre) + len(post)
                newl.extend(pre)
                newl.append(inst)
                newl.extend(post)
            if changed:
                bb.instructions = newl
    return n


def build_module(seq_len=4096, n_obs=N_OBS):
    nch = seq_len // P
    nit = (seq_len - 1 - UH) // (2 * UH)
    assert 2 * UH * nit + UH == seq_len - 1

    nc = Bass("TRN2", target_bir_lowering=False, debug=False, num_devices=8)

    emT_d = nc.dram_tensor("emT", [n_obs, N_STATES], _F32, kind="ExternalInput").ap()
    tr_d = nc.dram_tensor("tr", [N_STATES, N_STATES], _F32, kind="ExternalInput").ap()
    start_d = nc.dram_tensor("start", [SB, P], _F32, kind="ExternalInput").ap()
    obs_d = nc.dram_tensor("obs", [seq_len], _I32, kind="ExternalInput").ap()
    st_d = nc.dram_tensor("st", [seq_len + 1], _I32, kind="ExternalInput").ap()
    cb_d = nc.dram_tensor("cbias", [seq_len], _F32, kind="ExternalInput").ap()
    totc_d = nc.dram_tensor("totc", [1, 1], _F32, kind="ExternalInput").ap()
    s0f_d = nc.dram_tensor("s0f", [SB, 1], _F32, kind="ExternalInput").ap()
    out_d = nc.dram_tensor("out", [1], _F32, kind="ExternalOutput").ap()

    # on-device intermediate: eh table [p, t, b] = exp(em[t, 128b+p] - c_t)
    eh_d = nc.dram_tensor("ehtab", [P, seq_len, SB], _BF16).ap()

    with tile.TileContext(nc) as tc, ExitStack() as ctx:
        const = ctx.enter_context(tc.tile_pool(name="const", bufs=1))
        sbuf = ctx.enter_context(tc.tile_pool(name="sbuf", bufs=2))
        psum = ctx.enter_context(tc.tile_pool(name="psum", bufs=2, space="PSUM"))

        # ---------- constants ----------
        ident = const.tile([P, P], _F32)
        make_identity(nc, ident[:])
        identb = const.tile([P, P], _BF16)
        nc.vector.tensor_copy(out=identb[:], in_=ident[:])
        iota_s = const.tile([P, N_STATES], _I32)
        nc.gpsimd.iota(iota_s[:], pattern=[[1, N_STATES]], base=0,
                       channel_multiplier=0)
        iota_f = const.tile([P, N_STATES], _F32)
        nc.vector.tensor_copy(out=iota_f[:], in_=iota_s[:])
        # v-form iota on 8 partitions: value(b, k) = 128*b + k
        iotav_s = const.tile([SB, P], _I32)
        nc.gpsimd.iota(iotav_s[:], pattern=[[1, P]], base=0,
                       channel_multiplier=P)
        iotav_f = const.tile([SB, P], _F32)
        nc.vector.tensor_copy(out=iotav_f[:], in_=iotav_s[:])
        totc = const.tile([1, 1], _F32)
        nc.gpsimd.dma_start(totc[:], totc_d[:])
        s0f = const.tile([SB, 1], _F32)
        nc.gpsimd.dma_start(s0f[:], s0f_d[:])
        lbias = const.tile([SB, 1], _F32)
        nc.vector.memset(lbias[:], LOG1024)

        # index tiles [128, nch]: [p, c] = seq[128c + p]
        obs_sb = const.tile([P, nch], _I32)
        st_sb = const.tile([P, nch], _I32)
        st_next = const.tile([P, nch], _I32)
        cb_sb = const.tile([P, nch], _F32)
        nc.gpsimd.dma_start(obs_sb[:], obs_d.rearrange('(c p) -> p c', p=P))
        nc.gpsimd.dma_start(st_sb[:], st_d[0:seq_len].rearrange('(c p) -> p c', p=P))
        nc.gpsimd.dma_start(st_next[:],
                            st_d[1:seq_len + 1].rearrange('(c p) -> p c', p=P))
        nc.gpsimd.dma_start(cb_sb[:], cb_d.rearrange('(c p) -> p c', p=P))

        # ---------- E = exp(transition) as fp8 [p, ib, j] ----------
        E_sb = const.tile([P, SB, N_STATES], _FP8)
        for ib in range(SB):
            tt = sbuf.tile([P, N_STATES], _F32, tag="tload")
            nc.gpsimd.dma_start(tt[:], tr_d[P * ib:P * (ib + 1), :])
            te = sbuf.tile([P, N_STATES], _F32, tag="texp")
            nc.scalar.activation(out=te[:], in_=tt[:],
                                 func=mybir.ActivationFunctionType.Exp)
            nc.vector.tensor_copy(out=E_sb[:, ib, :], in_=te[:])

        # ---------- numerator accumulator ----------
        acc_num = const.tile([P, 1], _F32)
        nc.vector.memset(acc_num[:], 0.0)

        # start term: start[s0] added into partitions 0..7
        smask = const.tile([SB, P], _F32)
        start_sb = const.tile([SB, P], _F32)
        nc.gpsimd.dma_start(start_sb[:], start_d[:])
        nc.vector.tensor_tensor(out=smask[:], in0=iotav_f[:],
                                in1=s0f[:].to_broadcast([SB, P]),
                                op=mybir.AluOpType.is_equal)
        smr = const.tile([SB, P], _F32)
        nc.vector.tensor_mul(out=smr[:], in0=start_sb[:], in1=smask[:])
        sred = const.tile([SB, 1], _F32)
        nc.vector.reduce_sum(out=sred[:], in_=smr[:], axis=mybir.AxisListType.X)
        nc.vector.tensor_add(out=acc_num[0:SB, :], in0=acc_num[0:SB, :],
                             in1=sred[:])

        # ---------- prep chunks: emission gather -> em term + eh table ----------
        for c in range(nch):
            em_t = sbuf.tile([P, N_STATES], _F32, tag="em")
            nc.gpsimd.indirect_dma_start(
                out=em_t[:], out_offset=None, in_=emT_d[:],
                in_offset=bass.IndirectOffsetOnAxis(ap=obs_sb[:, c:c + 1], axis=0))
            stf = sbuf.tile([P, 1], _F32, tag="stf")
            nc.vector.tensor_copy(out=stf[:], in_=st_sb[:, c:c + 1])
            mask = sbuf.tile([P, N_STATES], _F32, tag="mask")
            nc.vector.tensor_tensor(out=mask[:], in0=iota_f[:],
                                    in1=stf[:].to_broadcast([P, N_STATES]),
                                    op=mybir.AluOpType.is_equal)
            mr = sbuf.tile([P, N_STATES], _F32, tag="mr")
            nc.vector.tensor_mul(out=mr[:], in0=em_t[:], in1=mask[:])
            mred = sbuf.tile([P, 1], _F32, tag="mred")
            nc.vector.reduce_sum(out=mred[:], in_=mr[:], axis=mybir.AxisListType.X)
            nc.vector.tensor_add(out=acc_num[:], in0=acc_num[:], in1=mred[:])
            ehf = sbuf.tile([P, N_STATES], _BF16, tag="ehf")
            nc.scalar.activation(out=ehf[:], in_=em_t[:],
                                 func=mybir.ActivationFunctionType.Exp,
                                 bias=cb_sb[:, c:c + 1])
            stg = sbuf.tile([P, P, SB], _BF16, tag="stg")
            for b in range(SB):
                tp = psum.tile([P, P], _BF16, tag="t1")
                nc.tensor.transpose(out=tp[:], in_=ehf[:, P * b:P * (b + 1)],
                                    identity=identb[:])
                nc.vector.tensor_copy(out=stg[:, :, b], in_=tp[:])
            nc.gpsimd.dma_start(eh_d[:, P * c:P * (c + 1), :], stg[:])

        # ---------- transition term ----------
        for c in range(nch):
            trr = sbuf.tile([P, N_STATES], _F32, tag="em")
            nc.gpsimd.indirect_dma_start(
                out=trr[:], out_offset=None, in_=tr_d[:],
                in_offset=bass.IndirectOffsetOnAxis(ap=st_sb[:, c:c + 1], axis=0))
            snf = sbuf.tile([P, 1], _F32, tag="stf")
            nc.vector.tensor_copy(out=snf[:], in_=st_next[:, c:c + 1])
            mask = sbuf.tile([P, N_STATES], _F32, tag="mask")
            nc.vector.tensor_tensor(out=mask[:], in0=iota_f[:],
                                    in1=snf[:].to_broadcast([P, N_STATES]),
                                    op=mybir.AluOpType.is_equal)
            mr = sbuf.tile([P, N_STATES], _F32, tag="mr")
            nc.vector.tensor_mul(out=mr[:], in0=trr[:], in1=mask[:])
            mred = sbuf.tile([P, 1], _F32, tag="mred")
            nc.vector.reduce_sum(out=mred[:], in_=mr[:], axis=mybir.AxisListType.X)
            nc.vector.tensor_add(out=acc_num[:], in0=acc_num[:], in1=mred[:])

        # ---------- v0 = 1024 * exp(start) * eh[0]  (fp8, v-form) ----------
        est = const.tile([SB, P], _F32)
        nc.scalar.activation(out=est[:], in_=start_sb[:],
                             func=mybir.ActivationFunctionType.Exp,
                             bias=lbias[:])
        v_a = const.tile([P, SB, 16], _FP8, tag="va")
        v_b = const.tile([P, SB, 16], _FP8, tag="vb")
        tp0 = psum.tile([P, SB], _F32, tag="t2")
        nc.tensor.transpose(out=tp0[:], in_=est[:], identity=ident[0:SB, 0:SB])
        eh0 = const.tile([P, SB], _BF16)
        nc.gpsimd.dma_start(eh0[:], eh_d[:, 0:1, :].rearrange('p a b -> p (a b)'))
        nc.vector.tensor_mul(out=v_a[:, :, 0], in0=tp0[:], in1=eh0[:])

        # ---------- scan ----------
        slot0 = const.tile([P, UH, SB], _BF16, tag="slot0")
        slot1 = const.tile([P, UH, SB], _BF16, tag="slot1")
        stA = const.tile([P, P], _BF16, tag="stA")
        stB = const.tile([P, P], _BF16, tag="stB")
        nc.vector.memset(stA[:], 0.0)
        nc.vector.memset(stB[:], 0.0)

        nc.gpsimd.dma_start(slot0[:], eh_d[:, 1:1 + UH, :])

        def step(u, slot, v_cur, v_nxt):
            mv = psum.tile([P, N_STATES], _F32, tag="mv")
            for h in range(2):
                for m in range(4):
                    nc.tensor.matmul(
                        out=mv[0:1, 512 * h:512 * (h + 1)],
                        lhsT=v_cur[:, 2 * m:2 * m + 2, 0:1],
                        rhs=E_sb[:, 2 * m:2 * m + 2, 512 * h:512 * (h + 1)],
                        start=(m == 0), stop=(m == 3),
                        perf_mode=mybir.MatmulPerfMode.DoubleRow,
                        skip_group_check=True)
            # partition-aligned assembly: block b -> stX[32*(b%4), :]
            for b in range(SB):
                stx = stA if b < 4 else stB
                src = mv[0:1, P * b:P * (b + 1)]
                dst = stx[32 * (b % 4):32 * (b % 4) + 1, :]
                if b % 2 == 0:
                    nc.vector.tensor_copy(out=dst, in_=src)
                else:
                    nc.scalar.copy(dst, src)
            t1 = psum.tile([P, P], _BF16, tag="t1")
            t2 = psum.tile([P, P], _BF16, tag="t2")
            nc.tensor.transpose(out=t1[:], in_=stA[:], identity=identb[:])
            nc.tensor.transpose(out=t2[:], in_=stB[:], identity=identb[:])
            # v block b lives in t1[:, 32b] (b<4) / t2[:, 32(b-4)]
            nc.vector.tensor_mul(out=v_nxt[:, 0:4, 0], in0=t1[:, 0:P:32],
                                 in1=slot[:, u, 0:4])
            nc.vector.tensor_mul(out=v_nxt[:, 4:SB, 0], in0=t2[:, 0:P:32],
                                 in1=slot[:, u, 4:SB])

        def half(slot):
            for u in range(UH):
                step(u, slot,
                     v_a if u % 2 == 0 else v_b,
                     v_b if u % 2 == 0 else v_a)

        eh_sh1 = eh_d[:, UH:, :]
        eh_sh2 = eh_d[:, 2 * UH:, :]
        with tc.For_i(1, 1 + 2 * UH * nit, 2 * UH) as i:
            nc.sync.dma_start(slot1[:], eh_sh1[:, bass.ds(i, UH), :])
            half(slot0)
            nc.sync.dma_start(slot0[:], eh_sh2[:, bass.ds(i, UH), :])
            half(slot1)
        half(slot0)  # epilogue steps (UH odd -> ends in v_b)

        v_fin = v_b
        # ---------- tail: log(sum(v)) + totc - num ----------
        vred = const.tile([P, 1], _F32)
        nc.vector.reduce_sum(out=vred[:], in_=v_fin[:, :, 0],
                             axis=mybir.AxisListType.X)
        den_ps = psum.tile([1, P], _F32, tag="t1")
        nc.tensor.transpose(out=den_ps[:], in_=vred[:], identity=ident[:])
        num_ps = psum.tile([1, P], _F32, tag="t2")
        nc.tensor.transpose(out=num_ps[:], in_=acc_num[:], identity=ident[:])
        den_s = const.tile([1, 1], _F32)
        nc.vector.reduce_sum(out=den_s[:], in_=den_ps[:], axis=mybir.AxisListType.X)
        num_s = const.tile([1, 1], _F32)
        nc.vector.reduce_sum(out=num_s[:], in_=num_ps[:], axis=mybir.AxisListType.X)
        logden = const.tile([1, 1], _F32)
        nc.scalar.activation(out=logden[:], in_=den_s[:],
                             func=mybir.ActivationFunctionType.Ln)
        res = const.tile([1, 1], _F32)
        # res = (logden + totc) - num
        nc.vector.scalar_tensor_tensor(
            out=res[:], in0=logden[:], scalar=totc[:], in1=num_s[:],
            op0=mybir.AluOpType.add, op1=mybir.AluOpType.subtract)
        nc.gpsimd.dma_start(out_d.rearrange('(a b) -> a b', b=1), res[:])

    _split_multi_sync(nc)
    return nc


def host_prep(start, transition, emission, obs_seq, state_seq):
    start = np.asarray(start, np.float32)
    transition = np.asarray(transition, np.float32)
    emission = np.asarray(emission, np.float32)
    obs_seq = np.asarray(obs_seq, np.int32)
    state_seq = np.asarray(state_seq, np.int32)

    # layout prep: transpose emission so per-timestep columns are contiguous
    # rows for the device-side indirect row gather
    emT = np.ascontiguousarray(emission.T)
    # per-timestep rescale estimate c_t = log(sum_j colmean(expT)_j * exp(em_t_j))
    cs = np.exp(transition, dtype=np.float64).mean(axis=0)
    em_rows = emT[obs_seq].astype(np.float64)          # [T, S]
    m0 = em_rows.max(axis=1, keepdims=True)
    c_t = (np.log(np.exp(em_rows - m0) @ cs) + m0[:, 0])
    totc = np.array([[c_t.sum() - np.log(1024.0)]], np.float32)

    return {
        "emT": emT,
        "tr": transition,
        "start": start.reshape(SB, P),
        "obs": obs_seq,
        "st": np.append(state_seq, np.int32(2000)).astype(np.int32),
        "cbias": (-c_t).astype(np.float32),
        "totc": totc,
        "s0f": np.full((SB, 1), float(state_seq[0]), np.float32),
    }


_CACHED = {}


def kernel(start, transition, emission, obs_seq, state_seq):
    in_map = host_prep(start, transition, emission, obs_seq, state_seq)
    if "nc" not in _CACHED:
        _CACHED["nc"] = build_module()
    nc = _CACHED["nc"]
    res = run_bass_kernel_spmd(nc, [in_map] * 8, list(range(8)))
    out = res.results[0]["out"]
    return np.float32(out.reshape(())[()])



# revision 8
# speedup vs baseline: 99.4764x; 99.4764x over previous
"""CRF NLL kernel for Trainium2 (8 NeuronCores, chunked rank-1 scan).

Math: the 4095-step forward recursion  alpha_t = logsumexp_i(alpha_{t-1}
+ T[:,j]) + em_t  is a product of positive matrices  M_t = exp(T)
diag(exp(em_t)).  Products of >=7 such matrices are rank-1 to machine
precision (Perron contraction per step ~0.006, measured sigma2/sigma1 ~
1e-15 at L=8), so the chain is cut into K=585 chunks of L=7 matrices:

    log_den = log(r_1 . q_2) + sum_{k=2..K-1} [log(p_k . q_{k+1}) - log s_k]

with p_k^T = 1^T P_k (forward vector scan of chunk k), q_k = P_k 1
(backward scan), s_k = 1^T P_k 1, r_1 = alpha_0^T P_1.  Using
m_k = p_k @ E and the pre-matmul backward state w (q = E w):
dot_k = m_k . w_{k+1} and s_k = colsum(E) . w_k.  Each core owns 73
forward chunks + the 73 shifted backward chunks, so every term is
core-local: no collectives; the host sums 8 partial scalars.

All 73 streams per direction advance in lockstep as ONE batched matvec
(lhsT [128, 2, 73] fp8 DoubleRow, rhs = E / E^T fp8 [128, 2, 512]):
matmul cost is independent of batch width.  Per step: 8 DR matmuls ->
PSUM [73, 1024] f32 -> eh-multiply in row form against gathered
emission rows (DVE half from PSUM; ACT copies the other half to SBUF
and GPSIMD multiplies it -- GPSIMD cannot touch PSUM) -> 8 PE
transposes back to v-form -> DVE/ACT requantize copy to fp8.  Forward
and backward interleave to keep every engine busy.  8+6 batched steps
replace 4095 serial steps.

Rescaling: one global constant cbar (est. log growth/step) keeps all
stream values O(1) for fp8; it cancels exactly in the final formula.
The numerator is 8 element-level indirect gathers sharded over cores.
Validated end-to-end vs fp32 reference in numpy: rel err ~2.7e-4.
"""
import sys

sys.path.insert(0, '/opt/trn_rl_repo')

from contextlib import ExitStack

import numpy as np

import concourse.bass as bass
import concourse.mybir as mybir
import concourse.tile as tile
from concourse.bass import Bass
from concourse.bass_utils import run_bass_kernel_spmd
from concourse.masks import make_identity

N_STATES = 1024
N_OBS = 32000
SEQ = 4096
P = 128
SB = 8
L = 7             # chunk length (matrices per chunk)
K = (SEQ - 1) // L            # 585 chunks
W = (K - 1) // 8              # 73 streams per direction per core
NCOL = L + (L - 1) + 2        # gather cols: fwd steps, bwd steps, init, em0

_F32 = mybir.dt.float32
_BF16 = mybir.dt.bfloat16
_FP8 = mybir.dt.float8e4
_I32 = mybir.dt.int32


def _split_multi_sync(nc):
    """This walrus build rejects >1 sync wait / update per instruction.
    Move extras onto same-engine NoOps (engine queues are in-order)."""
    n = 0
    for f in nc.m.functions:
        for bb in f.blocks:
            newl = []
            changed = False
            for inst in bb.instructions:
                si = inst.sync_info
                waits = list(si.on_wait or []) if si is not None else []
                updates = list(si.on_update or []) if si is not None else []
                pre = []
                post = []
                if len(waits) > 1:
                    for k, w in enumerate(waits[:-1]):
                        nop = mybir.InstNoOp(name=f"{inst.name}-wsp{k}",
                                             engine=inst.engine)
                        nop.sync_info = mybir.SyncInfo(on_wait=[w], on_update=[])
                        pre.append(nop)
                    waits = waits[-1:]
                if len(updates) > 1:
                    for k, u in enumerate(updates[1:]):
                        nop = mybir.InstNoOp(name=f"{inst.name}-usp{k}",
                                             engine=inst.engine)
                        nop.sync_info = mybir.SyncInfo(on_wait=[], on_update=[u])
                        post.append(nop)
                    updates = updates[:1]
                if pre or post:
                    changed = True
                    inst.sync_info = mybir.SyncInfo(on_wait=waits, on_update=updates)
                    n += len(pre) + len(post)
                newl.extend(pre)
                newl.append(inst)
                newl.extend(post)
            if changed:
                bb.instructions = newl
    return n


def build_module():
    nc = Bass("TRN2", target_bir_lowering=False, debug=False, num_devices=8)

    emT_d = nc.dram_tensor("emT", [N_OBS, N_STATES], _F32, kind="ExternalInput").ap()
    tr_d = nc.dram_tensor("tr", [N_STATES, N_STATES], _F32, kind="ExternalInput").ap()
    stf_d = nc.dram_tensor("stf", [N_STATES], _F32, kind="ExternalInput").ap()
    gidx_d = nc.dram_tensor("gidx", [P, NCOL], _I32, kind="ExternalInput").ap()
    eoff_d = nc.dram_tensor("eoff", [P, 4], _I32, kind="ExternalInput").ap()
    toff_d = nc.dram_tensor("toff", [P, 4], _I32, kind="ExternalInput").ap()
    nmask_d = nc.dram_tensor("nmask", [P, 8], _F32, kind="ExternalInput").ap()
    s0off_d = nc.dram_tensor("s0off", [P, 1], _I32, kind="ExternalInput").ap()
    zcol_d = nc.dram_tensor("zcol", [P, 1], _F32, kind="ExternalInput").ap()
    cbias_d = nc.dram_tensor("cbias", [P, 1], _F32, kind="ExternalInput").ap()
    smask_d = nc.dram_tensor("smask", [P, 1], _F32, kind="ExternalInput").ap()
    srow_d = nc.dram_tensor("srow", [1, N_STATES], _F32, kind="ExternalInput").ap()
    orow_d = nc.dram_tensor("orow", [1, N_STATES], _F32, kind="ExternalInput").ap()
    zsc_d = nc.dram_tensor("zsc", [1, 1], _F32, kind="ExternalInput").ap()
    out_d = nc.dram_tensor("out", [2], _F32, kind="ExternalOutput").ap()

    emT_flat = emT_d.rearrange('a (b c) -> (a b) c', c=1)
    tr_flat = tr_d.rearrange('a (b c) -> (a b) c', c=1)
    stf_2d = stf_d.rearrange('(a b) -> a b', b=1)

    with tile.TileContext(nc) as tc, ExitStack() as ctx:
        const = ctx.enter_context(tc.tile_pool(name="const", bufs=1))
        sbuf = ctx.enter_context(tc.tile_pool(name="sbuf", bufs=2))
        psum = ctx.enter_context(tc.tile_pool(name="psum", bufs=1, space="PSUM"))

        # ---------- constants / small inputs ----------
        ident = const.tile([P, P], _F32)
        make_identity(nc, ident[:])
        identb = const.tile([P, P], _BF16)
        nc.vector.tensor_copy(out=identb[:], in_=ident[:])
        ones32 = const.tile([P, 1], _F32)
        nc.vector.memset(ones32[:], 1.0)

        gidx = const.tile([P, NCOL], _I32)
        nc.gpsimd.dma_start(gidx[:], gidx_d[:])
        eoff = const.tile([P, 4], _I32)
        nc.sync.dma_start(eoff[:], eoff_d[:])
        toff = const.tile([P, 4], _I32)
        nc.sync.dma_start(toff[:], toff_d[:])
        nmask = const.tile([P, 8], _F32)
        nc.sync.dma_start(nmask[:], nmask_d[:])
        s0off = const.tile([P, 1], _I32)
        nc.sync.dma_start(s0off[:], s0off_d[:])
        zcol = const.tile([P, 1], _F32)
        nc.sync.dma_start(zcol[:], zcol_d[:])
        cbias = const.tile([P, 1], _F32)
        nc.sync.dma_start(cbias[:], cbias_d[:])
        smask = const.tile([P, 1], _F32)
        nc.sync.dma_start(smask[:], smask_d[:])
        srow = const.tile([1, N_STATES], _F32)
        nc.sync.dma_start(srow[:], srow_d[:])
        orow = const.tile([1, N_STATES], _F32)
        nc.sync.dma_start(orow[:], orow_d[:])
        zsc = const.tile([1, 1], _F32)
        nc.sync.dma_start(zsc[:], zsc_d[:])

        # ---------- emission gathers (row form, one col per scan step) ----
        # col c in [0, L): fwd step c+1 ; [L, 2L-1): bwd step c-L+1
        # col 2L-1: bwd init ; col 2L: em0 (2 rows)
        graw = []
        for c in range(NCOL):
            nrow = W if c < NCOL - 1 else 2
            g = sbuf.tile([nrow, N_STATES], _F32, tag=f"graw{c % 4}", bufs=4,
                          name=f"graw{c}")
            nc.gpsimd.indirect_dma_start(
                out=g[:], out_offset=None, in_=emT_d[:],
                in_offset=bass.IndirectOffsetOnAxis(ap=gidx[0:nrow, c:c + 1], axis=0))
            graw.append(g)

        # numerator element gathers
        ne = const.tile([P, 8], _F32)
        for c in range(4):
            nc.gpsimd.indirect_dma_start(
                out=ne[:, c:c + 1], out_offset=None, in_=emT_flat,
                in_offset=bass.IndirectOffsetOnAxis(ap=eoff[:, c:c + 1], axis=0))
        for c in range(4):
            nc.gpsimd.indirect_dma_start(
                out=ne[:, 4 + c:5 + c], out_offset=None, in_=tr_flat,
                in_offset=bass.IndirectOffsetOnAxis(ap=toff[:, c:c + 1], axis=0))
        s0g = const.tile([P, 1], _F32)
        nc.gpsimd.indirect_dma_start(
            out=s0g[:], out_offset=None, in_=stf_2d,
            in_offset=bass.IndirectOffsetOnAxis(ap=s0off[:], axis=0))

        # ---------- eh tables: exp(em - cbar) bf16, row form ----------
        eh = []
        for c in range(2 * L - 1):
            t = const.tile([W, N_STATES], _BF16, name=f"eh{c}")
            nc.scalar.activation(out=t[:], in_=graw[c][:],
                                 func=mybir.ActivationFunctionType.Exp,
                                 bias=cbias[0:W, :])
            eh.append(t)
        ehinit = const.tile([W, N_STATES], _BF16)
        nc.scalar.activation(out=ehinit[:], in_=graw[2 * L - 1][:],
                             func=mybir.ActivationFunctionType.Exp)

        # ---------- E = exp(T) fp8 [p, ib, j];  ET = E^T fp8 ----------
        E_sb = const.tile([P, SB, N_STATES], _FP8)
        ET_sb = const.tile([P, SB, N_STATES], _FP8)
        for ib in range(SB):
            tt = sbuf.tile([P, N_STATES], _F32, tag="tload", bufs=3, name=f"tld{ib}")
            nc.sync.dma_start(tt[:], tr_d[P * ib:P * (ib + 1), :])
            ebf = sbuf.tile([P, N_STATES], _BF16, tag="ebf", bufs=3, name=f"ebf{ib}")
            nc.scalar.activation(out=ebf[:], in_=tt[:],
                                 func=mybir.ActivationFunctionType.Exp)
            nc.vector.tensor_copy(out=E_sb[:, ib, 0:512], in_=ebf[:, 0:512])
            nc.gpsimd.tensor_copy(out=E_sb[:, ib, 512:1024], in_=ebf[:, 512:1024])
            # transpose this row-block into ET column slice
            etr = psum.tile([P, SB, P], _BF16, tag="etr", name=f"etr{ib}")
            for jb in range(SB):
                nc.tensor.transpose(out=etr[:, jb, :],
                                    in_=ebf[:, P * jb:P * (jb + 1)],
                                    identity=identb[:])
            nc.vector.tensor_copy(out=ET_sb[:, 0:4, P * ib:P * (ib + 1)],
                                  in_=etr[:, 0:4, :])
            nc.scalar.copy(ET_sb[:, 4:SB, P * ib:P * (ib + 1)],
                           etr[:, 4:SB, :])

        # ---------- r = colsum(E) in v-form f32 [128, 8, 1] ----------
        ones8 = const.tile([P, SB, 1], _FP8, padded_shape=[P, SB, 16])
        nc.vector.memset(ones8[:], 1.0)
        rps = psum.tile([1, N_STATES], _F32, tag="mm", bufs=2, name="rps")
        for h in range(2):
            for m in range(4):
                nc.tensor.matmul(
                    out=rps[0:1, 512 * h:512 * (h + 1)],
                    lhsT=ones8[:, 2 * m:2 * m + 2, :],
                    rhs=E_sb[:, 2 * m:2 * m + 2, 512 * h:512 * (h + 1)],
                    start=(m == 0), stop=(m == 3),
                    perf_mode=mybir.MatmulPerfMode.DoubleRow,
                    skip_group_check=True)
        rrow = const.tile([1, N_STATES], _BF16)
        nc.vector.tensor_copy(out=rrow[:], in_=rps[:])
        rtr = psum.tile([P, SB, W], _BF16, tag="tr", bufs=2,
                        padded_shape=[P, SB, P], name="rtr")
        for b in range(SB):
            nc.tensor.transpose(out=rtr[:, b, 0:1],
                                in_=rrow[0:1, P * b:P * (b + 1)],
                                identity=identb[0:1, 0:1])
        r_vf = const.tile([P, SB, 1], _F32)
        nc.vector.tensor_copy(out=r_vf[:], in_=rtr[:, :, 0:1])

        # ---------- stream inits ----------
        # fwd: ones; stream 0 (chunk 1, core 0 only) = exp(start + em0 - a0)
        fvf0 = sbuf.tile([P, SB, W], _FP8, tag="fvf", padded_shape=[P, SB, 80],
                         name="fvf0")
        nc.vector.memset(fvf0[:], 1.0)
        s1row = const.tile([1, N_STATES], _F32)
        nc.vector.tensor_add(out=s1row[:], in0=graw[2 * L][0:1, :], in1=srow[:])
        e0x = const.tile([1, N_STATES], _BF16)
        nc.scalar.activation(out=e0x[:], in_=s1row[:],
                             func=mybir.ActivationFunctionType.Exp)
        row0 = const.tile([1, N_STATES], _BF16)
        nc.vector.scalar_tensor_tensor(
            out=row0[:], in0=e0x[:], scalar=zsc[:], in1=orow[:],
            op0=mybir.AluOpType.mult, op1=mybir.AluOpType.add)
        e0tr = psum.tile([P, SB, W], _BF16, tag="tr", bufs=2,
                         padded_shape=[P, SB, P], name="e0tr")
        for b in range(SB):
            nc.tensor.transpose(out=e0tr[:, b, 0:1],
                                in_=row0[0:1, P * b:P * (b + 1)],
                                identity=identb[0:1, 0:1])
        nc.scalar.copy(fvf0[:, :, 0:1], e0tr[:, :, 0:1])
        # bwd: w = exp(em_b) from ehinit
        witr = psum.tile([P, SB, W], _BF16, tag="tr", bufs=2,
                         padded_shape=[P, SB, P], name="witr")
        for b in range(SB):
            nc.tensor.transpose(out=witr[:, b, :],
                                in_=ehinit[:, P * b:P * (b + 1)],
                                identity=identb[0:W, 0:W])
        wvf0 = sbuf.tile([P, SB, W], _FP8, tag="wvf", padded_shape=[P, SB, 80],
                         name="wvf0")
        nc.vector.tensor_copy(out=wvf0[:, 0:4, :], in_=witr[:, 0:4, :])
        nc.scalar.copy(wvf0[:, 4:SB, :], witr[:, 4:SB, :])

        # ---------- scan ----------
        fvf, wvf = fvf0, wvf0
        mvf = const.tile([P, SB, W], _BF16)

        def mm8(ps, v, rhs_tab):
            for h in range(2):
                for m in range(4):
                    nc.tensor.matmul(
                        out=ps[0:W, 512 * h:512 * (h + 1)],
                        lhsT=v[:, 2 * m:2 * m + 2, :],
                        rhs=rhs_tab[:, 2 * m:2 * m + 2, 512 * h:512 * (h + 1)],
                        start=(m == 0), stop=(m == 3),
                        perf_mode=mybir.MatmulPerfMode.DoubleRow,
                        skip_group_check=True)

        def tail(ps, ehsl, nxt, name=""):
            stg = sbuf.tile([W, N_STATES], _BF16, tag="stg", name=f"stg{name}")
            if ehsl is None:
                nc.vector.tensor_copy(out=stg[:, 0:512], in_=ps[0:W, 0:512])
                nc.scalar.copy(stg[:, 512:1024], ps[0:W, 512:1024])
            else:
                # h0: DVE multiplies straight from PSUM; h1: ACT copies to
                # SBUF (GPSIMD cannot read PSUM), GPSIMD multiplies there.
                nc.vector.tensor_mul(out=stg[:, 0:512], in0=ps[0:W, 0:512],
                                     in1=ehsl[:, 0:512])
                hcp = sbuf.tile([W, 512], _BF16, tag="hcp", name=f"hcp{name}")
                nc.scalar.copy(hcp[:], ps[0:W, 512:1024])
                nc.gpsimd.tensor_mul(out=stg[:, 512:1024], in0=hcp[:],
                                     in1=ehsl[:, 512:1024])
            tr = psum.tile([P, SB, W], _BF16, tag="tr", bufs=2,
                           padded_shape=[P, SB, P], name=f"tr{name}")
            for b in range(SB):
                nc.tensor.transpose(out=tr[:, b, :],
                                    in_=stg[0:W, P * b:P * (b + 1)],
                                    identity=identb[0:W, 0:W])
            nc.vector.tensor_copy(out=nxt[:, 0:4, :], in_=tr[:, 0:4, :])
            nc.scalar.copy(nxt[:, 4:SB, :], tr[:, 4:SB, :])

        for p in range(L + 1):
            i = p + 1             # fwd step 1..8 (8 = bare)
            s = p + 1             # bwd step 1..6
            psf = psum.tile([W, N_STATES], _F32, tag="mm", bufs=2,
                            padded_shape=[P, N_STATES], name=f"psf{i}")
            mm8(psf, fvf, E_sb)
            do_b = s <= L - 1
            if do_b:
                psb = psum.tile([W, N_STATES], _F32, tag="mm", bufs=2,
                                padded_shape=[P, N_STATES], name=f"psb{s}")
                mm8(psb, wvf, ET_sb)
            if i <= L:
                fnx = sbuf.tile([P, SB, W], _FP8, tag="fvf", padded_shape=[P, SB, 80],
                                name=f"fvf{i}")
                tail(psf, eh[i - 1][:], fnx, name=f"f{i}")
                fvf = fnx
            else:
                tail(psf, None, mvf, name="m")
            if do_b:
                wnx = sbuf.tile([P, SB, W], _FP8, tag="wvf", padded_shape=[P, SB, 80],
                                name=f"wvf{s}")
                tail(psb, eh[L + s - 1][:], wnx, name=f"b{s}")
                wvf = wnx

        # ---------- tail: dots, s, logs, numerator ----------
        pr = const.tile([P, SB, W], _F32)
        nc.vector.tensor_mul(out=pr[:], in0=mvf[:], in1=wvf[:])
        spr = const.tile([P, SB, W], _F32)
        nc.vector.tensor_mul(out=spr[:], in0=wvf[:],
                             in1=r_vf[:].to_broadcast([P, SB, W]))
        tailps = psum.tile([W, 8], _F32, tag="tail", padded_shape=[P, 512],
                           name="tailps")
        for b in range(SB):
            nc.tensor.matmul(out=tailps[0:W, 0:1], lhsT=pr[:, b, :],
                             rhs=ones32[:], start=(b == 0), stop=(b == SB - 1),
                             skip_group_check=True)
        for b in range(SB):
            nc.tensor.matmul(out=tailps[0:W, 1:2], lhsT=spr[:, b, :],
                             rhs=ones32[:], start=(b == 0), stop=(b == SB - 1),
                             skip_group_check=True)
        ld = const.tile([W, 1], _F32)
        nc.scalar.activation(out=ld[:], in_=tailps[0:W, 0:1],
                             func=mybir.ActivationFunctionType.Ln)
        ls = const.tile([W, 1], _F32)
        nc.scalar.activation(out=ls[:], in_=tailps[0:W, 1:2],
                             func=mybir.ActivationFunctionType.Ln)
        # d = ld + smask * ls   (smask = -1 include / 0 drop)
        d34 = const.tile([W, 1], _F32)
        nc.vector.scalar_tensor_tensor(
            out=d34[:], in0=ls[:], scalar=smask[0:W, :], in1=ld[:],
            op0=mybir.AluOpType.mult, op1=mybir.AluOpType.add)
        nc.tensor.matmul(out=tailps[0:1, 4:5], lhsT=d34[:], rhs=ones32[0:W, :])
        # numerator partial
        nem = const.tile([P, 8], _F32)
        nc.vector.tensor_mul(out=nem[:], in0=ne[:], in1=nmask[:])
        nred = const.tile([P, 1], _F32)
        nc.vector.reduce_sum(out=nred[:], in_=nem[:], axis=mybir.AxisListType.X)
        nvec = const.tile([P, 1], _F32)
        nc.vector.scalar_tensor_tensor(
            out=nvec[:], in0=s0g[:], scalar=zcol[:], in1=nred[:],
            op0=mybir.AluOpType.mult, op1=mybir.AluOpType.add)
        nc.tensor.matmul(out=tailps[0:1, 5:6], lhsT=nvec[:], rhs=ones32[:])
        o_sb = const.tile([1, 2], _F32)
        nc.vector.tensor_copy(out=o_sb[:], in_=tailps[0:1, 4:6])
        nc.sync.dma_start(out_d.rearrange('(a b) -> a b', a=1), o_sb[:])

    _split_multi_sync(nc)
    return nc


def host_prep(start, transition, emission, obs_seq, state_seq):
    start = np.asarray(start, np.float32)
    T = np.asarray(transition, np.float32)
    emission = np.asarray(emission, np.float32)
    obs = np.asarray(obs_seq, np.int64)
    st = np.asarray(state_seq, np.int64)

    emT = np.ascontiguousarray(emission.T)          # [N_OBS, N_STATES]
    em64 = emT[obs[:64]]
    cbar = np.float32(np.log(1024.) + T.mean() + T.var() / 2 + em64.mean())
    a0 = np.float32(start.mean() + emT[obs[0]].mean())

    maps = []
    for m in range(8):
        gidx = np.zeros((P, NCOL), np.int32)
        for i in range(1, L + 1):                   # fwd step cols
            for j in range(W):
                gidx[j, i - 1] = obs[(W * m + j) * L + i]
        for s in range(1, L):                       # bwd step cols
            for j in range(W):
                gidx[j, L + s - 1] = obs[(W * m + 2 + j) * L - s]
        gidx[0:W, 2 * L - 1] = [obs[(W * m + 2 + j) * L] for j in range(W)]
        gidx[0:2, 2 * L] = obs[0]
        # numerator offsets (this core's 512 timesteps)
        ts = np.arange(512 * m, 512 * m + 512)
        eoff = (obs[ts] * 1024 + st[ts]).astype(np.int32).reshape(4, P).T.copy()
        tp = ts.copy()
        tmask = np.ones(512, np.float32)
        if m == 7:
            tp[-1] = 0
            tmask[-1] = 0.0
        toff = (st[tp] * 1024 + st[np.minimum(tp + 1, SEQ - 1)]).astype(
            np.int32).reshape(4, P).T.copy()
        nmask = np.ones((P, 8), np.float32)
        nmask[:, 4:8] = tmask.reshape(4, P).T
        smaskv = np.full((P, 1), -1.0, np.float32)
        smaskv[W:, 0] = 0.0
        if m == 7:
            smaskv[W - 1, 0] = 0.0                  # chunk K has no s-term
        zcol = np.zeros((P, 1), np.float32)
        if m == 0:
            zcol[0, 0] = 1.0
        maps.append({
            "emT": emT,
            "tr": T,
            "stf": start,
            "gidx": gidx,
            "eoff": eoff,
            "toff": toff,
            "nmask": nmask,
            "s0off": np.full((P, 1), int(st[0]), np.int32),
            "zcol": zcol,
            "cbias": np.full((P, 1), -cbar, np.float32),
            "smask": smaskv,
            "srow": (start - a0).reshape(1, -1) if m == 0
                    else np.zeros((1, N_STATES), np.float32),
            "orow": np.zeros((1, N_STATES), np.float32) if m == 0
                    else np.ones((1, N_STATES), np.float32),
            "zsc": np.array([[1.0 if m == 0 else 0.0]], np.float32),
        })
    return maps, float(cbar), float(a0)


_CACHED = {}


def kernel(start, transition, emission, obs_seq, state_seq):
    maps, cbar, a0 = host_prep(start, transition, emission, obs_seq, state_seq)
    if "nc" not in _CACHED:
        _CACHED["nc"] = build_module()
    nc = _CACHED["nc"]
    res = run_bass_kernel_spmd(nc, maps, list(range(8)))
    tot = 0.0
    for m in range(8):
        o = np.asarray(res.results[m]["out"], np.float64).reshape(2)
        tot += o[0] - o[1]
    tot += a0 + (L * K - 1) * np.float64(np.float32(cbar))
    return np.float32(tot)


# revision 11
# speedup vs baseline: 110.4472x; 1.1103x over previous
"""CRF NLL kernel for Trainium2 (8 NeuronCores, chunked rank-1 scan).

Math: the 4095-step forward recursion  alpha_t = logsumexp_i(alpha_{t-1}
+ T[:,j]) + em_t  is a product of positive matrices  M_t = exp(T)
diag(exp(em_t)).  Products of >=7 such matrices are rank-1 to machine
precision (Perron contraction per step ~0.006, measured sigma2/sigma1 ~
1e-15 at L=8), so the chain is cut into K=585 chunks of L=7 matrices:

    log_den = log(r_1 . q_2) + sum_{k=2..K-1} [log(p_k . q_{k+1}) - log s_k]

with p_k^T = 1^T P_k (forward vector scan of chunk k), q_k = P_k 1
(backward scan), s_k = 1^T P_k 1, r_1 = alpha_0^T P_1.  Using
m_k = p_k @ E and the pre-matmul backward state w (q = E w):
dot_k = m_k . w_{k+1} and s_k = colsum(E) . w_k.  Each core owns 73
forward chunks + the 73 shifted backward chunks, so every term is
core-local: no collectives; the host sums 8 partial scalars.

All 73 streams per direction advance in lockstep as ONE batched matvec
(lhsT [128, 2, 73] fp8 DoubleRow, rhs = E / E^T fp8 [128, 2, 512]):
matmul cost is independent of batch width.  Per step: 8 DR matmuls ->
PSUM [73, 1024] f32 -> eh-multiply in row form against gathered
emission rows (DVE half from PSUM; ACT copies the other half to SBUF
and GPSIMD multiplies it -- GPSIMD cannot touch PSUM) -> 8 PE
transposes back to v-form -> DVE/ACT requantize copy to fp8.  Forward
and backward interleave to keep every engine busy.  8+6 batched steps
replace 4095 serial steps.

Rescaling: one global constant cbar (est. log growth/step) keeps all
stream values O(1) for fp8; it cancels exactly in the final formula.
The numerator is 8 element-level indirect gathers sharded over cores.
Validated end-to-end vs fp32 reference in numpy: rel err ~2.7e-4.
"""
import sys

sys.path.insert(0, '/opt/trn_rl_repo')

from contextlib import ExitStack

import numpy as np

import concourse.bass as bass
import concourse.mybir as mybir
import concourse.tile as tile
from concourse.bass import Bass
from concourse.bass_utils import run_bass_kernel_spmd
from concourse.masks import make_identity

N_STATES = 1024
N_OBS = 32000
SEQ = 4096
P = 128
SB = 8
L = 7             # chunk length (matrices per chunk)
K = (SEQ - 1) // L            # 585 chunks
W = (K - 1) // 8              # 73 streams per direction per core
NCOL = L + (L - 1) + 1        # gather cols: fwd steps, bwd steps, init

_F32 = mybir.dt.float32
_BF16 = mybir.dt.bfloat16
_FP8 = mybir.dt.float8e4
_I32 = mybir.dt.int32


def _split_multi_sync(nc):
    """This walrus build rejects >1 sync wait / update per instruction.
    Move extras onto same-engine NoOps (engine queues are in-order)."""
    n = 0
    for f in nc.m.functions:
        for bb in f.blocks:
            newl = []
            changed = False
            for inst in bb.instructions:
                si = inst.sync_info
                waits = list(si.on_wait or []) if si is not None else []
                updates = list(si.on_update or []) if si is not None else []
                pre = []
                post = []
                if len(waits) > 1:
                    for k, w in enumerate(waits[:-1]):
                        nop = mybir.InstNoOp(name=f"{inst.name}-wsp{k}",
                                             engine=inst.engine)
                        nop.sync_info = mybir.SyncInfo(on_wait=[w], on_update=[])
                        pre.append(nop)
                    waits = waits[-1:]
                if len(updates) > 1:
                    for k, u in enumerate(updates[1:]):
                        nop = mybir.InstNoOp(name=f"{inst.name}-usp{k}",
                                             engine=inst.engine)
                        nop.sync_info = mybir.SyncInfo(on_wait=[], on_update=[u])
                        post.append(nop)
                    updates = updates[:1]
                if pre or post:
                    changed = True
                    inst.sync_info = mybir.SyncInfo(on_wait=waits, on_update=updates)
                    n += len(pre) + len(post)
                newl.extend(pre)
                newl.append(inst)
                newl.extend(post)
            if changed:
                bb.instructions = newl
    return n


def build_module():
    nc = Bass("TRN2", target_bir_lowering=False, debug=False, num_devices=8)

    emT_d = nc.dram_tensor("emT", [N_OBS, N_STATES], _F32, kind="ExternalInput").ap()
    tr_d = nc.dram_tensor("tr", [N_STATES, N_STATES], _F32, kind="ExternalInput").ap()
    stf_d = nc.dram_tensor("stf", [N_STATES], _F32, kind="ExternalInput").ap()
    gidx_d = nc.dram_tensor("gidx", [P, NCOL], _I32, kind="ExternalInput").ap()
    eoff_d = nc.dram_tensor("eoff", [P, 4], _I32, kind="ExternalInput").ap()
    toff_d = nc.dram_tensor("toff", [P, 4], _I32, kind="ExternalInput").ap()
    nmask_d = nc.dram_tensor("nmask", [P, 8], _F32, kind="ExternalInput").ap()
    s0off_d = nc.dram_tensor("s0off", [P, 1], _I32, kind="ExternalInput").ap()
    zcol_d = nc.dram_tensor("zcol", [P, 1], _F32, kind="ExternalInput").ap()
    cbias_d = nc.dram_tensor("cbias", [P, 1], _F32, kind="ExternalInput").ap()
    smask_d = nc.dram_tensor("smask", [P, 1], _F32, kind="ExternalInput").ap()
    sev_d = nc.dram_tensor("sev", [P, SB], _F32, kind="ExternalInput").ap()
    zv_d = nc.dram_tensor("zv", [P, 1], _F32, kind="ExternalInput").ap()
    ov_d = nc.dram_tensor("ov", [P, 1], _F32, kind="ExternalInput").ap()
    out_d = nc.dram_tensor("out", [2], _F32, kind="ExternalOutput").ap()

    emT_flat = emT_d.rearrange('a (b c) -> (a b) c', c=1)  # noqa
    tr_flat = tr_d.rearrange('a (b c) -> (a b) c', c=1)
    stf_2d = stf_d.rearrange('(a b) -> a b', b=1)

    with tile.TileContext(nc) as tc, ExitStack() as ctx:
        const = ctx.enter_context(tc.tile_pool(name="const", bufs=1))
        sbuf = ctx.enter_context(tc.tile_pool(name="sbuf", bufs=2))
        psum = ctx.enter_context(tc.tile_pool(name="psum", bufs=1, space="PSUM"))

        # ---------- constants / small inputs ----------
        ident = const.tile([P, P], _F32)
        make_identity(nc, ident[:])
        identb = const.tile([P, P], _BF16)
        nc.vector.tensor_copy(out=identb[:], in_=ident[:])
        ones32 = const.tile([P, 1], _F32)
        nc.vector.memset(ones32[:], 1.0)

        gidx = const.tile([P, NCOL], _I32)
        nc.sync.dma_start(gidx[:], gidx_d[:])
        eoff = const.tile([P, 4], _I32)
        nc.sync.dma_start(eoff[:], eoff_d[:])
        toff = const.tile([P, 4], _I32)
        nc.sync.dma_start(toff[:], toff_d[:])
        nmask = const.tile([P, 8], _F32)
        nc.sync.dma_start(nmask[:], nmask_d[:])
        s0off = const.tile([P, 1], _I32)
        nc.sync.dma_start(s0off[:], s0off_d[:])
        zcol = const.tile([P, 1], _F32)
        nc.sync.dma_start(zcol[:], zcol_d[:])
        cbias = const.tile([P, 1], _F32)
        nc.sync.dma_start(cbias[:], cbias_d[:])
        smask = const.tile([P, 1], _F32)
        nc.sync.dma_start(smask[:], smask_d[:])
        sev = const.tile([P, SB], _F32)
        nc.sync.dma_start(sev[:], sev_d[:])
        zv = const.tile([P, 1], _F32)
        nc.sync.dma_start(zv[:], zv_d[:])
        ov = const.tile([P, 1], _F32)
        nc.sync.dma_start(ov[:], ov_d[:])

        # ---------- emission gathers (row form, one col per scan step) ----
        # col c in [0, L): fwd step c+1 ; [L, 2L-1): bwd step c-L+1 ;
        # col 2L-1: bwd init.  Gather order puts each column on the gpsimd
        # SWDGE queue just before its consumer needs it: fwd steps, bwd
        # init, bwd steps, then the numerator elements.
        graw = [None] * NCOL
        def gath(c):
            g = sbuf.tile([W, N_STATES], _F32, tag=f"graw{c % 4}", bufs=4,
                          name=f"graw{c}")
            nc.gpsimd.indirect_dma_start(
                out=g[:], out_offset=None, in_=emT_d[:],
                in_offset=bass.IndirectOffsetOnAxis(ap=gidx[0:W, c:c + 1], axis=0))
            graw[c] = g
        for c in range(L):
            gath(c)
        gath(2 * L - 1)
        for c in range(L, 2 * L - 1):
            gath(c)

        # numerator element gathers
        ne = const.tile([P, 8], _F32)
        for c in range(4):
            nc.gpsimd.indirect_dma_start(
                out=ne[:, c:c + 1], out_offset=None, in_=emT_flat,
                in_offset=bass.IndirectOffsetOnAxis(ap=eoff[:, c:c + 1], axis=0))
        for c in range(4):
            nc.gpsimd.indirect_dma_start(
                out=ne[:, 4 + c:5 + c], out_offset=None, in_=tr_flat,
                in_offset=bass.IndirectOffsetOnAxis(ap=toff[:, c:c + 1], axis=0))
        s0g = const.tile([P, 1], _F32)
        nc.gpsimd.indirect_dma_start(
            out=s0g[:], out_offset=None, in_=stf_2d,
            in_offset=bass.IndirectOffsetOnAxis(ap=s0off[:], axis=0))

        # ---------- eh tables: exp(em - cbar) bf16, row form ----------
        eh = []
        for c in range(2 * L - 1):
            t = const.tile([W, N_STATES], _BF16, name=f"eh{c}")
            nc.scalar.activation(out=t[:], in_=graw[c][:],
                                 func=mybir.ActivationFunctionType.Exp,
                                 bias=cbias[0:W, :])
            eh.append(t)
        ehinit = const.tile([W, N_STATES], _BF16)
        nc.scalar.activation(out=ehinit[:], in_=graw[2 * L - 1][:],
                             func=mybir.ActivationFunctionType.Exp)

        # ---------- E = exp(T) fp8 [p, ib, j];  ET = E^T fp8 ----------
        E_sb = const.tile([P, SB, N_STATES], _FP8)
        ET_sb = const.tile([P, SB, N_STATES], _FP8)
        for ib in range(SB):
            tt = sbuf.tile([P, N_STATES], _F32, tag="tload", bufs=3, name=f"tld{ib}")
            nc.sync.dma_start(tt[:], tr_d[P * ib:P * (ib + 1), :])
            ebf = sbuf.tile([P, N_STATES], _BF16, tag="ebf", bufs=3, name=f"ebf{ib}")
            nc.scalar.activation(out=ebf[:], in_=tt[:],
                                 func=mybir.ActivationFunctionType.Exp)
            nc.vector.tensor_copy(out=E_sb[:, ib, 0:512], in_=ebf[:, 0:512])
            nc.scalar.copy(E_sb[:, ib, 512:1024], ebf[:, 512:1024])
            # transpose this row-block into ET column slice
            etr = psum.tile([P, SB, P], _BF16, tag="etr", name=f"etr{ib}")
            for jb in range(SB):
                nc.tensor.transpose(out=etr[:, jb, :],
                                    in_=ebf[:, P * jb:P * (jb + 1)],
                                    identity=identb[:])
            nc.vector.tensor_copy(out=ET_sb[:, 0:4, P * ib:P * (ib + 1)],
                                  in_=etr[:, 0:4, :])
            nc.scalar.copy(ET_sb[:, 4:SB, P * ib:P * (ib + 1)],
                           etr[:, 4:SB, :])

        # ---------- r = colsum(E) in v-form f32 [128, 8, 1] ----------
        ones8 = const.tile([P, SB, 1], _FP8, padded_shape=[P, SB, 16])
        nc.vector.memset(ones8[:], 1.0)
        rps = psum.tile([1, N_STATES], _F32, tag="mm", bufs=2, name="rps")
        for h in range(2):
            for m in range(4):
                nc.tensor.matmul(
                    out=rps[0:1, 512 * h:512 * (h + 1)],
                    lhsT=ones8[:, 2 * m:2 * m + 2, :],
                    rhs=E_sb[:, 2 * m:2 * m + 2, 512 * h:512 * (h + 1)],
                    start=(m == 0), stop=(m == 3),
                    perf_mode=mybir.MatmulPerfMode.DoubleRow,
                    skip_group_check=True)
        rrow = const.tile([1, N_STATES], _BF16)
        nc.vector.tensor_copy(out=rrow[:], in_=rps[:])
        rtr = psum.tile([P, SB, W], _BF16, tag="tr", bufs=2,
                        padded_shape=[P, SB, P], name="rtr")
        for b in range(SB):
            nc.tensor.transpose(out=rtr[:, b, 0:1],
                                in_=rrow[0:1, P * b:P * (b + 1)],
                                identity=identb[0:1, 0:1])
        r_vf = const.tile([P, SB, 1], _F32)
        nc.vector.tensor_copy(out=r_vf[:], in_=rtr[:, :, 0:1])

        # ---------- stream inits ----------
        # fwd: ones; stream 0 (chunk 1, core 0 only) = exp(start + em0 - a0)
        fvf0 = sbuf.tile([P, SB, W], _FP8, tag="fvf", padded_shape=[P, SB, 80],
                         name="fvf0")
        nc.vector.memset(fvf0[:], 1.0)
        e0x = const.tile([P, SB], _F32)
        nc.scalar.activation(out=e0x[:], in_=sev[:],
                             func=mybir.ActivationFunctionType.Exp)
        e0z = const.tile([P, SB], _F32)
        nc.vector.tensor_mul(out=e0z[:], in0=e0x[:],
                             in1=zv[:].to_broadcast([P, SB]))
        nc.vector.tensor_add(out=fvf0[:, :, 0], in0=e0z[:],
                             in1=ov[:].to_broadcast([P, SB]))
        # bwd: w = exp(em_b) from ehinit
        witr = psum.tile([P, SB, W], _BF16, tag="tr", bufs=2,
                         padded_shape=[P, SB, P], name="witr")
        for b in range(SB):
            nc.tensor.transpose(out=witr[:, b, :],
                                in_=ehinit[:, P * b:P * (b + 1)],
                                identity=identb[0:W, 0:W])
        wvf0 = sbuf.tile([P, SB, W], _FP8, tag="wvf", padded_shape=[P, SB, 80],
                         name="wvf0")
        nc.vector.tensor_copy(out=wvf0[:, 0:4, :], in_=witr[:, 0:4, :])
        nc.scalar.copy(wvf0[:, 4:SB, :], witr[:, 4:SB, :])

        # ---------- scan ----------
        fvf, wvf = fvf0, wvf0
        mvf = const.tile([P, SB, W], _BF16)

        def mm8(ps, v, rhs_tab):
            for h in range(2):
                for m in range(4):
                    nc.tensor.matmul(
                        out=ps[0:W, 512 * h:512 * (h + 1)],
                        lhsT=v[:, 2 * m:2 * m + 2, :],
                        rhs=rhs_tab[:, 2 * m:2 * m + 2, 512 * h:512 * (h + 1)],
                        start=(m == 0), stop=(m == 3),
                        perf_mode=mybir.MatmulPerfMode.DoubleRow,
                        skip_group_check=True)

        def tail(ps, ehsl, nxt, name="", h1dve=False):
            stg = sbuf.tile([W, N_STATES], _BF16, tag="stg", name=f"stg{name}")
            if ehsl is None:
                nc.vector.tensor_copy(out=stg[:, 0:512], in_=ps[0:W, 0:512])
                nc.scalar.copy(stg[:, 512:1024], ps[0:W, 512:1024])
            else:
                # h0: DVE multiplies straight from PSUM; h1: ACT copies to
                # SBUF (GPSIMD cannot read PSUM) then GPSIMD or DVE
                # multiplies there (split by direction to balance load).
                nc.vector.tensor_mul(out=stg[:, 0:512], in0=ps[0:W, 0:512],
                                     in1=ehsl[:, 0:512])
                hcp = sbuf.tile([W, 512], _BF16, tag="hcp", name=f"hcp{name}")
                nc.scalar.copy(hcp[:], ps[0:W, 512:1024])
                eng = nc.vector if h1dve else nc.gpsimd
                eng.tensor_mul(out=stg[:, 512:1024], in0=hcp[:],
                               in1=ehsl[:, 512:1024])
            tr = psum.tile([P, SB, W], _BF16, tag="tr", bufs=2,
                           padded_shape=[P, SB, P], name=f"tr{name}")
            for b in range(SB):
                nc.tensor.transpose(out=tr[:, b, :],
                                    in_=stg[0:W, P * b:P * (b + 1)],
                                    identity=identb[0:W, 0:W])
            nc.vector.tensor_copy(out=nxt[:, 0:4, :], in_=tr[:, 0:4, :])
            nc.scalar.copy(nxt[:, 4:SB, :], tr[:, 4:SB, :])

        for p in range(L + 1):
            i = p + 1             # fwd step 1..8 (8 = bare)
            s = p + 1             # bwd step 1..6
            psf = psum.tile([W, N_STATES], _F32, tag="mm", bufs=2,
                            padded_shape=[P, N_STATES], name=f"psf{i}")
            mm8(psf, fvf, E_sb)
            do_b = s <= L - 1
            if do_b:
                psb = psum.tile([W, N_STATES], _F32, tag="mm", bufs=2,
                                padded_shape=[P, N_STATES], name=f"psb{s}")
                mm8(psb, wvf, ET_sb)
            if i <= L:
                fnx = sbuf.tile([P, SB, W], _FP8, tag="fvf", padded_shape=[P, SB, 80],
                                name=f"fvf{i}")
                tail(psf, eh[i - 1][:], fnx, name=f"f{i}")
                fvf = fnx
            else:
                tail(psf, None, mvf, name="m")
            if do_b:
                wnx = sbuf.tile([P, SB, W], _FP8, tag="wvf", padded_shape=[P, SB, 80],
                                name=f"wvf{s}")
                tail(psb, eh[L + s - 1][:], wnx, name=f"b{s}", h1dve=True)
                wvf = wnx

        # ---------- tail: dots, s, logs, numerator ----------
        pr = const.tile([P, SB, W], _F32)
        nc.vector.tensor_mul(out=pr[:], in0=mvf[:], in1=wvf[:])
        spr = const.tile([P, SB, W], _F32)
        nc.vector.tensor_mul(out=spr[:], in0=wvf[:],
                             in1=r_vf[:].to_broadcast([P, SB, W]))
        tailps = psum.tile([W, 8], _F32, tag="tail", padded_shape=[P, 512],
                           name="tailps")
        for b in range(SB):
            nc.tensor.matmul(out=tailps[0:W, 0:1], lhsT=pr[:, b, :],
                             rhs=ones32[:], start=(b == 0), stop=(b == SB - 1),
                             skip_group_check=True)
        for b in range(SB):
            nc.tensor.matmul(out=tailps[0:W, 1:2], lhsT=spr[:, b, :],
                             rhs=ones32[:], start=(b == 0), stop=(b == SB - 1),
                             skip_group_check=True)
        ld = const.tile([W, 1], _F32)
        nc.scalar.activation(out=ld[:], in_=tailps[0:W, 0:1],
                             func=mybir.ActivationFunctionType.Ln)
        ls = const.tile([W, 1], _F32)
        nc.scalar.activation(out=ls[:], in_=tailps[0:W, 1:2],
                             func=mybir.ActivationFunctionType.Ln)
        # d = ld + smask * ls   (smask = -1 include / 0 drop)
        lsm = const.tile([W, 1], _F32)
        nc.vector.tensor_mul(out=lsm[:], in0=ls[:], in1=smask[0:W, :])
        d34 = const.tile([W, 1], _F32)
        nc.vector.tensor_add(out=d34[:], in0=ld[:], in1=lsm[:])
        nc.tensor.matmul(out=tailps[0:1, 4:5], lhsT=d34[:], rhs=ones32[0:W, :])
        # numerator partial
        nem = const.tile([P, 8], _F32)
        nc.vector.tensor_mul(out=nem[:], in0=ne[:], in1=nmask[:])
        nred = const.tile([P, 1], _F32)
        nc.vector.reduce_sum(out=nred[:], in_=nem[:], axis=mybir.AxisListType.X)
        s0z = const.tile([P, 1], _F32)
        nc.vector.tensor_mul(out=s0z[:], in0=s0g[:], in1=zcol[:])
        nvec = const.tile([P, 1], _F32)
        nc.vector.tensor_add(out=nvec[:], in0=s0z[:], in1=nred[:])
        nc.tensor.matmul(out=tailps[0:1, 5:6], lhsT=nvec[:], rhs=ones32[:])
        o_sb = const.tile([1, 2], _F32)
        nc.vector.tensor_copy(out=o_sb[:], in_=tailps[0:1, 4:6])
        nc.sync.dma_start(out_d.rearrange('(a b) -> a b', a=1), o_sb[:])

    _split_multi_sync(nc)
    return nc


def host_prep(start, transition, emission, obs_seq, state_seq):
    start = np.asarray(start, np.float32)
    T = np.asarray(transition, np.float32)
    emission = np.asarray(emission, np.float32)
    obs = np.asarray(obs_seq, np.int64)
    st = np.asarray(state_seq, np.int64)

    emT = np.ascontiguousarray(emission.T)          # [N_OBS, N_STATES]
    em64 = emT[obs[:64]]
    cbar = np.float32(np.log(1024.) + T.mean() + T.var() / 2 + em64.mean())
    a0 = np.float32(start.mean() + emT[obs[0]].mean())

    maps = []
    for m in range(8):
        gidx = np.zeros((P, NCOL), np.int32)
        for i in range(1, L + 1):                   # fwd step cols
            for j in range(W):
                gidx[j, i - 1] = obs[(W * m + j) * L + i]
        for s in range(1, L):                       # bwd step cols
            for j in range(W):
                gidx[j, L + s - 1] = obs[(W * m + 2 + j) * L - s]
        gidx[0:W, 2 * L - 1] = [obs[(W * m + 2 + j) * L] for j in range(W)]
        sev = ((start - a0 + emT[obs[0]]).reshape(SB, P).T.copy()
               if m == 0 else np.zeros((P, SB), np.float32))
        # numerator offsets (this core's 512 timesteps)
        ts = np.arange(512 * m, 512 * m + 512)
        eoff = (obs[ts] * 1024 + st[ts]).astype(np.int32).reshape(4, P).T.copy()
        tp = ts.copy()
        tmask = np.ones(512, np.float32)
        if m == 7:
            tp[-1] = 0
            tmask[-1] = 0.0
        toff = (st[tp] * 1024 + st[np.minimum(tp + 1, SEQ - 1)]).astype(
            np.int32).reshape(4, P).T.copy()
        nmask = np.ones((P, 8), np.float32)
        nmask[:, 4:8] = tmask.reshape(4, P).T
        smaskv = np.full((P, 1), -1.0, np.float32)
        smaskv[W:, 0] = 0.0
        if m == 7:
            smaskv[W - 1, 0] = 0.0                  # chunk K has no s-term
        zcol = np.zeros((P, 1), np.float32)
        if m == 0:
            zcol[0, 0] = 1.0
        maps.append({
            "emT": emT,
            "tr": T,
            "stf": start,
            "gidx": gidx,
            "eoff": eoff,
            "toff": toff,
            "nmask": nmask,
            "s0off": np.full((P, 1), int(st[0]), np.int32),
            "zcol": zcol,
            "cbias": np.full((P, 1), -cbar, np.float32),
            "smask": smaskv,
            "sev": sev.astype(np.float32),
            "zv": np.full((P, 1), 1.0 if m == 0 else 0.0, np.float32),
            "ov": np.full((P, 1), 0.0 if m == 0 else 1.0, np.float32),
        })
    return maps, float(cbar), float(a0)


_CACHED = {}


def kernel(start, transition, emission, obs_seq, state_seq):
    maps, cbar, a0 = host_prep(start, transition, emission, obs_seq, state_seq)
    if "nc" not in _CACHED:
        _CACHED["nc"] = build_module()
    nc = _CACHED["nc"]
    res = run_bass_kernel_spmd(nc, maps, list(range(8)))
    tot = 0.0
    for m in range(8):
        o = np.asarray(res.results[m]["out"], np.float64).reshape(2)
        tot += o[0] - o[1]
    tot += a0 + (L * K - 1) * np.float64(np.float32(cbar))
    return np.float32(tot)
